# revision 2
# baseline (speedup 1.0000x reference)
"""Trainium2 Bass kernel for nn_MultiHeadMixer.

Reference computation (B=4, S=2048, E=1024, H=16, D=64):
    xp = x @ inp_w.T + inp_b                      # (B,S,E)
    xh[b,h,d,s] = xp[b,s,h*D+d]
    y0[b,h,d,t] = sum_{s<=t} xh[b,h,d,s] * weight[h,t-s]   # causal Toeplitz
    y2 = y0 / cumsum(weight)[h,t] + bias[h,t]
    out[b,t,:] = (y2 reshaped to (E,)) @ out_w.T

Sharding (8 cores): core c = (batch b=c//2, head-group hg=c%2 of 8 heads,
embed cols 512*hg..512*hg+512).  Each core computes a full-(E) partial of
out[b].T; host sums the two head-group partials per batch and transposes.

On-device layout: everything runs in the "transposed" domain [feature, seq]:
  proj1:  xp[s,c]   = sum_e xT[e,s] * w1T[e,c]           (PE, K=e)
  mixer:  y0[d,t]   = sum_s xp[s,d] * Toeplitz[s,t]      (PE, K=s, per head)
  proj2:  outT[e',t]= sum_c w2T[c,e'] * y2[c,t]          (PE, K=c)
The causal Toeplitz matmul uses per-head strip tiles Tp[p, kap] =
weight[h, kap-384-p] so every (s-block i, t-quad q) block of the Toeplitz
matrix is a contiguous 512-column slice of one SBUF tile.
"""

import numpy as np
import ml_dtypes

import concourse.bass as bass
import concourse.bacc as bacc
import concourse.mybir as mybir
import concourse.tile as tile
from concourse.bass_utils import run_bass_kernel_spmd

B, S, E, H = 4, 2048, 1024, 16
D = E // H
N_CORES = 8
HPC = 8          # heads per core
CPC = 512        # embed cols per core
SB = S // 128    # 16 s-blocks
EB = E // 128    # 8 e-blocks
TQ = S // 512    # 4 t-quads
KAP = 2432       # Toeplitz strip width: 3 zero-blocks + 2048

BF16 = mybir.dt.bfloat16
F32 = mybir.dt.float32
NPBF16 = ml_dtypes.bfloat16

_CACHED = {}


def build_program():
    nc = bacc.Bacc("TRN2", target_bir_lowering=False, debug=False,
                   num_devices=N_CORES)

    xT = nc.dram_tensor("xT", [E, S], BF16, kind="ExternalInput").ap()
    w1T = nc.dram_tensor("w1T", [E, CPC], BF16, kind="ExternalInput").ap()
    b1x = nc.dram_tensor("b1x", [128, CPC], F32, kind="ExternalInput").ap()
    w2T = nc.dram_tensor("w2T", [CPC, E], BF16, kind="ExternalInput").ap()
    Tp = nc.dram_tensor("Tp", [HPC, 128, KAP], BF16, kind="ExternalInput").ap()
    invnx = nc.dram_tensor("invnx", [4, 128, S], F32, kind="ExternalInput").ap()
    biasx = nc.dram_tensor("biasx", [4, 128, S], F32, kind="ExternalInput").ap()
    outT = nc.dram_tensor("outT", [E, S], F32, kind="ExternalOutput").ap()

    with tile.TileContext(nc) as tc:
        with (
            tc.tile_pool(name="xt", bufs=EB) as xt_pool,
            tc.tile_pool(name="w1", bufs=EB) as w1_pool,
            tc.tile_pool(name="w2", bufs=4) as w2_pool,
            tc.tile_pool(name="tp", bufs=HPC) as tp_pool,
            tc.tile_pool(name="cn", bufs=8) as cn_pool,
            tc.tile_pool(name="b1", bufs=1) as b1_pool,
            tc.tile_pool(name="xp", bufs=SB) as xp_pool,
            tc.tile_pool(name="y2", bufs=4) as y2_pool,
            tc.tile_pool(name="tmp", bufs=3) as tmp_pool,
            tc.tile_pool(name="ost", bufs=4) as ost_pool,
            tc.tile_pool(name="ps1", bufs=2, space="PSUM") as ps1_pool,
            tc.tile_pool(name="psm", bufs=3, space="PSUM") as psm_pool,
            tc.tile_pool(name="ps2", bufs=2, space="PSUM") as ps2_pool,
        ):
            # ---- input loads ----
            xt_t = []
            w1_t = []
            for k in range(EB):
                t = xt_pool.tile([128, S], BF16, tag="xt")
                nc.sync.dma_start(t[:], xT[128 * k:128 * (k + 1), :])
                xt_t.append(t)
                w = w1_pool.tile([128, CPC], BF16, tag="w1")
                nc.sync.dma_start(w[:], w1T[128 * k:128 * (k + 1), :])
                w1_t.append(w)
            b1_t = b1_pool.tile([128, CPC], F32, tag="b1")
            nc.sync.dma_start(b1_t[:], b1x[:])

            tp_t = []
            for h in range(HPC):
                t = tp_pool.tile([128, KAP], BF16, tag="tp")
                nc.sync.dma_start(t[:], Tp[h])
                tp_t.append(t)
            invn_t, bias_t = [], []
            for hp in range(4):
                t = cn_pool.tile([128, S], F32, tag="cn")
                nc.sync.dma_start(t[:], invnx[hp])
                invn_t.append(t)
                t = cn_pool.tile([128, S], F32, tag="cn")
                nc.sync.dma_start(t[:], biasx[hp])
                bias_t.append(t)
            w2_t = []
            for k in range(4):
                t = w2_pool.tile([128, E], BF16, tag="w2")
                nc.sync.dma_start(t[:], w2T[128 * k:128 * (k + 1), :])
                w2_t.append(t)

            # ---- proj1: xp[s-blk][128, 512] ----
            xp_t = []
            for m in range(SB):
                ps = ps1_pool.tile([128, CPC], F32, tag="ps1")
                for k in range(EB):
                    nc.tensor.matmul(
                        ps[:],
                        xt_t[k][:, 128 * m:128 * (m + 1)],
                        w1_t[k][:],
                        start=(k == 0),
                        stop=(k == EB - 1),
                    )
                xp = xp_pool.tile([128, CPC], BF16, tag="xp")
                nc.vector.tensor_add(xp[:], ps[:], b1_t[:])
                xp_t.append(xp)

            # ---- mixer: y2 per head-pair [128, 2048] bf16 ----
            # Each (head-pair, t-quad) PSUM tile holds head 2hp on partitions
            # 0-63 and head 2hp+1 on 64-127 (auto col-tiling via out base
            # partition), so the normalize+bias epilogue runs on full tiles.
            y2_t = [y2_pool.tile([128, S], BF16, tag="y2", name=f"y2_{i}")
                    for i in range(4)]
            for hp in range(4):
                for q in range(TQ):
                    ps = psm_pool.tile([128, CPC], F32, tag="psm")
                    n_i = 4 * q + 4
                    for par in range(2):
                        h = 2 * hp + par
                        prow = slice(64 * par, 64 * par + 64)
                        for i in range(n_i):
                            off = 128 * (4 * q - i + 3)
                            nc.tensor.matmul(
                                ps[prow, :],
                                xp_t[i][:, 64 * h:64 * (h + 1)],
                                tp_t[h][:, off:off + CPC],
                                start=(i == 0),
                                stop=(i == n_i - 1),
                            )
                    tcol = slice(512 * q, 512 * (q + 1))
                    tmp = tmp_pool.tile([128, CPC], F32, tag="tmp")
                    nc.vector.tensor_mul(tmp[:], ps[:], invn_t[hp][:, tcol])
                    nc.vector.tensor_add(y2_t[hp][:, tcol], tmp[:],
                                         bias_t[hp][:, tcol])

            # ---- proj2: outT[e'-blk, t-quad] ----
            for n in range(EB):
                for q in range(TQ):
                    ps = ps2_pool.tile([128, CPC], F32, tag="ps2")
                    for k in range(4):
                        nc.tensor.matmul(
                            ps[:],
                            w2_t[k][:, 128 * n:128 * (n + 1)],
                            y2_t[k][:, 512 * q:512 * (q + 1)],
                            start=(k == 0),
                            stop=(k == 3),
                        )
                    ost = ost_pool.tile([128, CPC], F32, tag="ost")
                    nc.scalar.copy(ost[:], ps[:])
                    nc.sync.dma_start(
                        outT[128 * n:128 * (n + 1), 512 * q:512 * (q + 1)],
                        ost[:],
                    )

    nc.compile()
    return nc


def host_prep(x, weight, bias, inp_w, inp_b, out_w):
    """Build the 8 per-core input maps (host-side shard + layout prep)."""
    x = np.asarray(x, np.float32)
    weight = np.asarray(weight, np.float32)
    bias = np.asarray(bias, np.float32)
    inp_w = np.asarray(inp_w, np.float32)
    inp_b = np.asarray(inp_b, np.float32)
    out_w = np.asarray(out_w, np.float32)

    invn = 1.0 / np.cumsum(weight, axis=1)

    xT_b = [np.ascontiguousarray(x[b].T).astype(NPBF16) for b in range(B)]

    hg_pack = []
    for hg in range(2):
        heads = range(HPC * hg, HPC * hg + HPC)
        cols = slice(CPC * hg, CPC * hg + CPC)
        w1T = np.ascontiguousarray(inp_w[cols, :].T).astype(NPBF16)
        b1x = np.broadcast_to(inp_b[cols], (128, CPC)).astype(np.float32).copy()
        w2T = np.ascontiguousarray(out_w[:, cols].T).astype(NPBF16)
        Tp = np.zeros((HPC, 128, KAP), NPBF16)
        for hi, h in enumerate(heads):
            wrow = weight[h]
            for p in range(128):
                Tp[hi, p, 384 + p:KAP] = wrow[:2048 - p]
        invnx = np.zeros((4, 128, S), np.float32)
        biasx = np.zeros((4, 128, S), np.float32)
        for hp in range(4):
            h0 = HPC * hg + 2 * hp
            invnx[hp, :64] = invn[h0]
            invnx[hp, 64:] = invn[h0 + 1]
            biasx[hp, :64] = bias[h0]
            biasx[hp, 64:] = bias[h0 + 1]
        hg_pack.append(dict(w1T=w1T, b1x=b1x, w2T=w2T, Tp=Tp,
                            invnx=invnx, biasx=biasx))

    in_maps = []
    for c in range(N_CORES):
        b, hg = c // 2, c % 2
        m = dict(hg_pack[hg])
        m["xT"] = xT_b[b]
        in_maps.append(m)
    return in_maps


def kernel(x, weight, bias, inp_w, inp_b, out_w):
    if "nc" not in _CACHED:
        _CACHED["nc"] = build_program()
    nc = _CACHED["nc"]

    in_maps = host_prep(x, weight, bias, inp_w, inp_b, out_w)
    res = run_bass_kernel_spmd(nc, in_maps, core_ids=list(range(N_CORES)))

    out = np.empty((B, S, E), np.float32)
    for b in range(B):
        out[b] = (res.results[2 * b]["outT"] + res.results[2 * b + 1]["outT"]).T
    return out
